# revision 12
# baseline (speedup 1.0000x reference)
"""Trainium2 Bass kernel for nn_Attn (B=32, S=4096, H=1024, D=2*H=2048).

Reference computation:
    tmp      = einsum("bsd,hd->bsh", encoder_outputs, W) + b      # [B,S,H]
    energies = einsum("bh,bsh->bs", hidden, tmp)                  # [B,S]
    attn     = softmax(energies, axis=-1)[:, None, :]             # [B,1,S]

Key reassociation (exact in real arithmetic):
    energies[b,s] = enc[b,s,:] . v[b,:] + (hidden[b] . bias)
    with v[b,:] = hidden[b,:] @ W        # [B, D]
The bias term is constant over s, so it cancels inside softmax and is
dropped entirely.  This turns a 550-GFLOP dense matmul problem into a
memory-bound weighted-reduction stream over the 1 GiB encoder_outputs.

Sharding: data-parallel over batch across 8 cores (4 batches per core);
W replicated.  Each core:
  1. v = hidden_loc @ W via TensorE (hiddenT pre-transposed on host);
     W lives in a tightly-scoped pool so its 8 MiB of SBUF is reclaimed
     by the deep streaming pool right after the matmuls,
  2. flattens v to one partition (tiny SBUF->SBUF DMA) and broadcasts
     each v[b] to 128 partitions with ones-matmuls (no HBM traffic),
  3. streams enc tiles [128 s-partitions x SJ x 2048 d] (a small
     independent pool lets the first chunks stream during step 1/2)
     and reduces them on DVE with fused scalar_tensor_tensor
     (out = in0 * in1, accum_out = row-sum) against the broadcast v,
  4. gathers energies to a [4, 4096] batch-per-partition layout
     (SBUF->SBUF DMA) and does a standard stable softmax there,
  5. writes attn [4, 4096] back.
"""

import numpy as np

import concourse.bacc as bacc
import concourse.tile as tile
from concourse import mybir
from concourse.bass_utils import run_bass_kernel_spmd

F32 = mybir.dt.float32

B, S, H, D = 32, 4096, 1024, 2048
NCORES = 8
BL = B // NCORES          # batches per core = 4
KT = H // 128             # hidden k-tiles = 8
NJ = D // 512             # 512-wide N chunks in D for matmuls = 4
SJ = 2                    # s-rows per partition per streamed DMA chunk
NQ = S // (128 * SJ)      # streamed DMA chunks per batch = 16
SCOLS = S // 128          # energy columns per partition = 32
EARLY_CHUNKS = 2          # chunks streamed via the small independent pool
MAIN_BUFS = 7             # deep prefetch once the W pool is released


def build_bass():
    nc = bacc.Bacc()
    hT = nc.dram_tensor("hT", [128, KT * BL], F32, kind="ExternalInput")
    enc = nc.dram_tensor("enc", [BL, S, D], F32, kind="ExternalInput")
    W = nc.dram_tensor("W", [H, D], F32, kind="ExternalInput")
    out = nc.dram_tensor("out", [BL, S], F32, kind="ExternalOutput")

    # s = p*SCOLS + q*SJ + j   (p = partition, column c = q*SJ + j)
    enc_r = enc[:, :, :].rearrange("b (p q j) d -> b q p j d", p=128, q=NQ, j=SJ)

    with tile.TileContext(nc) as tc:
        with tc.tile_pool(name="persist", bufs=1) as persist:
            hT_sb = persist.tile([128, KT * BL], F32, tag="hT")
            nc.sync.dma_start(out=hT_sb, in_=hT[:, :])
            ones = persist.tile([1, 128], F32, tag="ones")
            nc.vector.memset(ones, 1.0)
            v_bc = [
                persist.tile([128, D], F32, tag=f"vb{b}", name=f"vb{b}")
                for b in range(BL)
            ]
            e_tiles = [
                persist.tile([128, SCOLS], F32, tag=f"e{b}", name=f"e{b}")
                for b in range(BL)
            ]
            es = persist.tile([BL, S], F32, tag="es")

            stt_kwargs = dict(
                scalar=1.0,
                op0=mybir.AluOpType.mult,
                op1=mybir.AluOpType.mult,
            )

            def load_chunk(pool, b, q):
                t = pool.tile([128, SJ, D], F32, tag="enc", name="enc_t")
                nc.sync.dma_start(out=t, in_=enc_r[b, q])
                return t

            def reduce_chunk(t, b, q):
                for j in range(SJ):
                    # Fused multiply + add-reduce on DVE in one pass:
                    # out = (in0 * 1.0) * in1, accum_out = sum(out).
                    # out aliases in0 (the product is dead after the reduce).
                    # NB: tensor_tensor_reduce wedges the device on this
                    # runtime path; scalar_tensor_tensor is the plain
                    # TENSOR_SCALAR_PTR ISA op and works.
                    nc.vector.scalar_tensor_tensor(
                        out=t[:, j, :],
                        in0=t[:, j, :],
                        in1=v_bc[b],
                        accum_out=e_tiles[b][:, q * SJ + j:q * SJ + j + 1],
                        **stt_kwargs,
                    )

            def stream_chunk(pool, b, q):
                reduce_chunk(load_chunk(pool, b, q), b, q)

            # Small independent pool: first chunks stream while W loads.
            with tc.tile_pool(name="early", bufs=EARLY_CHUNKS) as early:
                # ---- v = hidden_loc @ W; W pool is tightly scoped ----
                with (
                    tc.tile_pool(name="wpool", bufs=1) as wpool,
                    tc.tile_pool(name="psum_v", bufs=1, space="PSUM") as psum_v_pool,
                ):
                    w_sb = []
                    for k in range(KT):
                        wt = wpool.tile([128, D], F32, tag=f"w{k}", name=f"w{k}")
                        nc.sync.dma_start(out=wt, in_=W[k * 128:(k + 1) * 128, :])
                        w_sb.append(wt)
                    # early enc chunk DMAs issue right after the W DMAs; the
                    # consuming DVE ops are traced only after v_bc is
                    # written (below) so Tile sees the RAW dependency.
                    early_tiles = [
                        load_chunk(early, c // NQ, c % NQ)
                        for c in range(EARLY_CHUNKS)
                    ]

                    psum_v = psum_v_pool.tile([BL, D], F32, tag="psv")
                    for k in range(KT):
                        for j in range(NJ):
                            nc.tensor.matmul(
                                psum_v[:, j * 512:(j + 1) * 512],
                                hT_sb[:, k * BL:(k + 1) * BL],
                                w_sb[k][:, j * 512:(j + 1) * 512],
                                start=(k == 0),
                                stop=(k == KT - 1),
                            )

                # ---- flatten v to partition 0, broadcast via ones-matmul ----
                with (
                    tc.tile_pool(name="vpool", bufs=1) as vpool,
                    tc.tile_pool(name="psum_bc", bufs=4, space="PSUM") as psum_bc,
                ):
                    v_sb = vpool.tile([BL, D], F32, tag="v")
                    nc.scalar.copy(out=v_sb, in_=psum_v)
                    vflat = vpool.tile([1, BL * D], F32, tag="vflat")
                    nc.sync.dma_start(out=vflat, in_=v_sb)
                    for b in range(BL):
                        for j in range(NJ):
                            pb = psum_bc.tile([128, 512], F32, tag="pb", name="pb")
                            nc.tensor.matmul(
                                pb,
                                ones,
                                vflat[0:1, b * D + j * 512:b * D + (j + 1) * 512],
                                start=True,
                                stop=True,
                            )
                            # alternate copy engine: Scalar & Vector are both
                            # idle here; halves the serial copy chain
                            eng = nc.scalar if j % 2 == 0 else nc.vector
                            if eng is nc.scalar:
                                eng.copy(
                                    out=v_bc[b][:, j * 512:(j + 1) * 512], in_=pb
                                )
                            else:
                                eng.tensor_copy(
                                    v_bc[b][:, j * 512:(j + 1) * 512], pb
                                )

                # consume the early chunks now that v_bc exists
                for c, t in enumerate(early_tiles):
                    reduce_chunk(t, c // NQ, c % NQ)

                # ---- main stream: reuses the released W/v SBUF ----
                with tc.tile_pool(name="stream", bufs=MAIN_BUFS) as stream:
                    for c in range(EARLY_CHUNKS, BL * NQ):
                        stream_chunk(stream, c // NQ, c % NQ)

            # ---- gather energies into [BL, S] (batch-per-partition) ----
            for b in range(BL):
                nc.sync.dma_start(out=es[b:b + 1, :], in_=e_tiles[b][:, :])

            # ---- softmax along free dim ----
            mx = persist.tile([BL, 1], F32, tag="mx")
            nc.vector.tensor_reduce(
                out=mx, in_=es, axis=mybir.AxisListType.X, op=mybir.AluOpType.max
            )
            nmx = persist.tile([BL, 1], F32, tag="nmx")
            nc.scalar.mul(out=nmx, in_=mx, mul=-1.0)
            ssum = persist.tile([BL, 1], F32, tag="ssum")
            nc.scalar.activation(
                out=es,
                in_=es,
                func=mybir.ActivationFunctionType.Exp,
                bias=nmx,
                scale=1.0,
                accum_out=ssum,
            )
            rsum = persist.tile([BL, 1], F32, tag="rsum")
            nc.vector.reciprocal(out=rsum, in_=ssum)
            nc.vector.tensor_scalar_mul(es, es, rsum)
            nc.sync.dma_start(out=out[:, :], in_=es)

    nc.compile()
    return nc


_NC_CACHE = None


def _get_nc():
    global _NC_CACHE
    if _NC_CACHE is None:
        _NC_CACHE = build_bass()
    return _NC_CACHE


def _make_in_maps(hidden, encoder_outputs, W):
    hidden = np.asarray(hidden, dtype=np.float32)
    encoder_outputs = np.asarray(encoder_outputs, dtype=np.float32)
    W = np.ascontiguousarray(np.asarray(W, dtype=np.float32))
    in_maps = []
    for c in range(NCORES):
        hs = hidden[c * BL:(c + 1) * BL]                       # [BL, H]
        # hT[p, k*BL + b] = hs[b, k*128 + p]
        hT = np.ascontiguousarray(
            hs.T.reshape(KT, 128, BL).transpose(1, 0, 2).reshape(128, KT * BL)
        )
        in_maps.append({
            "hT": hT,
            "enc": np.ascontiguousarray(encoder_outputs[c * BL:(c + 1) * BL]),
            "W": W,
        })
    return in_maps


def run_device(hidden, encoder_outputs, W, trace=False, **spmd_kwargs):
    nc = _get_nc()
    in_maps = _make_in_maps(hidden, encoder_outputs, W)
    res = run_bass_kernel_spmd(
        nc, in_maps, core_ids=list(range(NCORES)), trace=trace, **spmd_kwargs
    )
    outs = np.concatenate([r["out"] for r in res.results], axis=0)  # [B, S]
    return outs[:, None, :].astype(np.float32), res


def kernel(hidden, encoder_outputs, W, b):
    # `b` (the Linear bias) shifts every energy in a row equally
    # (hidden[b].bias, independent of s), so it cancels in the softmax.
    out, _ = run_device(hidden, encoder_outputs, W)
    return out


# revision 13
# speedup vs baseline: 1.2398x; 1.2398x over previous
"""Trainium2 Bass kernel for nn_Attn (B=32, S=4096, H=1024, D=2*H=2048).

Reference computation:
    tmp      = einsum("bsd,hd->bsh", encoder_outputs, W) + b      # [B,S,H]
    energies = einsum("bh,bsh->bs", hidden, tmp)                  # [B,S]
    attn     = softmax(energies, axis=-1)[:, None, :]             # [B,1,S]

Key reassociation (exact in real arithmetic):
    energies[b,s] = enc[b,s,:] . v[b,:] + (hidden[b] . bias)
    with v[b,:] = hidden[b,:] @ W        # [B, D]
The bias term is constant over s, so it cancels inside softmax and is
dropped entirely.  This turns a 550-GFLOP dense matmul problem into a
memory-bound weighted-reduction stream over the 1 GiB encoder_outputs.

Sharding: data-parallel over batch across 8 cores (4 batches per core);
W replicated.  Each core:
  1. v = hidden_loc @ W via TensorE (hiddenT pre-transposed on host),
     k-contiguous so the chain finishes right as the W DMAs land,
  2. broadcasts v[b] to 128 partitions via a DRAM-roundtrip broadcast
     DMA (a partition-stride-0 read of a DRAM scratch row),
  3. streams enc tiles [128 s-partitions x SJ x 2048 d] and reduces
     them on DVE with fused scalar_tensor_tensor
     (out = in0 * in1, accum_out = row-sum) against the broadcast v,
  4. gathers energies to a [4, 4096] batch-per-partition layout
     (SBUF->SBUF DMA) and does a standard stable softmax there,
  5. writes attn [4, 4096] back.
"""

import numpy as np

import concourse.bacc as bacc
import concourse.tile as tile
from concourse import mybir
from concourse.bass_utils import run_bass_kernel_spmd

F32 = mybir.dt.float32

B, S, H, D = 32, 4096, 1024, 2048
NCORES = 8
BL = B // NCORES          # batches per core = 4
KT = H // 128             # hidden k-tiles = 8
NJ = D // 512             # 512-wide N chunks in D for matmuls = 4
SJ = 2                    # s-rows per partition per streamed DMA chunk
NQ = S // (128 * SJ)      # streamed DMA chunks per batch = 16
SCOLS = S // 128          # energy columns per partition = 32
STREAM_BUFS = 4


def build_bass():
    nc = bacc.Bacc()
    hT = nc.dram_tensor("hT", [128, KT * BL], F32, kind="ExternalInput")
    enc = nc.dram_tensor("enc", [BL, S, D], F32, kind="ExternalInput")
    W = nc.dram_tensor("W", [H, D], F32, kind="ExternalInput")
    out = nc.dram_tensor("out", [BL, S], F32, kind="ExternalOutput")

    with tile.TileContext(nc) as tc:
        with (
            tc.tile_pool(name="persist", bufs=1) as persist,
            tc.tile_pool(name="stream", bufs=STREAM_BUFS) as stream,
            tc.tile_pool(name="psum_v", bufs=1, space="PSUM") as psum_v_pool,
            tc.tile_pool(name="dram", bufs=1, space="DRAM") as dram_pool,
        ):
            # ---- load hiddenT and W ----
            hT_sb = persist.tile([128, KT * BL], F32, tag="hT")
            nc.sync.dma_start(out=hT_sb, in_=hT[:, :])

            w_sb = []
            for k in range(KT):
                wt = persist.tile([128, D], F32, tag=f"w{k}", name=f"w{k}")
                nc.sync.dma_start(out=wt, in_=W[k * 128:(k + 1) * 128, :])
                w_sb.append(wt)

            # ---- v = hidden_loc @ W  -> psum [BL, D] (k-contiguous) ----
            psum_v = psum_v_pool.tile([BL, D], F32, tag="psv")
            for k in range(KT):
                for j in range(NJ):
                    nc.tensor.matmul(
                        psum_v[:, j * 512:(j + 1) * 512],
                        hT_sb[:, k * BL:(k + 1) * BL],
                        w_sb[k][:, j * 512:(j + 1) * 512],
                        start=(k == 0),
                        stop=(k == KT - 1),
                    )
            v_sb = persist.tile([BL, D], F32, tag="v")
            nc.scalar.copy(out=v_sb, in_=psum_v)

            # ---- broadcast v[b] to 128 partitions via DRAM roundtrip ----
            vdram = dram_pool.tile([BL, D], F32, tag="vdram")
            nc.sync.dma_start(out=vdram, in_=v_sb)
            v_bc = []
            for b in range(BL):
                vb = persist.tile([128, D], F32, tag=f"vb{b}", name=f"vb{b}")
                nc.sync.dma_start(out=vb, in_=vdram[b:b + 1, :].to_broadcast([128, D]))
                v_bc.append(vb)

            # ---- stream enc, fused multiply + row-reduce on DVE ----
            # s = p*SCOLS + q*SJ + j   (p = partition, column c = q*SJ + j)
            enc_r = enc[:, :, :].rearrange(
                "b (p q j) d -> b q p j d", p=128, q=NQ, j=SJ
            )
            e_tiles = [
                persist.tile([128, SCOLS], F32, tag=f"e{b}", name=f"e{b}")
                for b in range(BL)
            ]
            for b in range(BL):
                for q in range(NQ):
                    t = stream.tile([128, SJ, D], F32, tag="enc", name="enc_t")
                    nc.sync.dma_start(out=t, in_=enc_r[b, q])
                    for j in range(SJ):
                        # Fused multiply + add-reduce on DVE in one pass:
                        # out = (in0 * 1.0) * in1, accum_out = sum(out).
                        # out aliases in0 (the product is dead after the
                        # reduce).  NB: tensor_tensor_reduce wedges the device
                        # on this runtime path; scalar_tensor_tensor is the
                        # plain TENSOR_SCALAR_PTR ISA op and works.
                        nc.vector.scalar_tensor_tensor(
                            out=t[:, j, :],
                            in0=t[:, j, :],
                            scalar=1.0,
                            in1=v_bc[b],
                            op0=mybir.AluOpType.mult,
                            op1=mybir.AluOpType.mult,
                            accum_out=e_tiles[b][:, q * SJ + j:q * SJ + j + 1],
                        )

            # ---- gather energies into [BL, S] (batch-per-partition) ----
            es = persist.tile([BL, S], F32, tag="es")
            for b in range(BL):
                nc.sync.dma_start(out=es[b:b + 1, :], in_=e_tiles[b][:, :])

            # ---- softmax along free dim ----
            mx = persist.tile([BL, 1], F32, tag="mx")
            nc.vector.tensor_reduce(
                out=mx, in_=es, axis=mybir.AxisListType.X, op=mybir.AluOpType.max
            )
            nmx = persist.tile([BL, 1], F32, tag="nmx")
            nc.scalar.mul(out=nmx, in_=mx, mul=-1.0)
            ssum = persist.tile([BL, 1], F32, tag="ssum")
            nc.scalar.activation(
                out=es,
                in_=es,
                func=mybir.ActivationFunctionType.Exp,
                bias=nmx,
                scale=1.0,
                accum_out=ssum,
            )
            rsum = persist.tile([BL, 1], F32, tag="rsum")
            nc.vector.reciprocal(out=rsum, in_=ssum)
            nc.vector.tensor_scalar_mul(es, es, rsum)
            nc.sync.dma_start(out=out[:, :], in_=es)

    nc.compile()
    return nc


_NC_CACHE = None


def _get_nc():
    global _NC_CACHE
    if _NC_CACHE is None:
        _NC_CACHE = build_bass()
    return _NC_CACHE


def _make_in_maps(hidden, encoder_outputs, W):
    hidden = np.asarray(hidden, dtype=np.float32)
    encoder_outputs = np.asarray(encoder_outputs, dtype=np.float32)
    W = np.ascontiguousarray(np.asarray(W, dtype=np.float32))
    in_maps = []
    for c in range(NCORES):
        hs = hidden[c * BL:(c + 1) * BL]                       # [BL, H]
        # hT[p, k*BL + b] = hs[b, k*128 + p]
        hT = np.ascontiguousarray(
            hs.T.reshape(KT, 128, BL).transpose(1, 0, 2).reshape(128, KT * BL)
        )
        in_maps.append({
            "hT": hT,
            "enc": np.ascontiguousarray(encoder_outputs[c * BL:(c + 1) * BL]),
            "W": W,
        })
    return in_maps


def run_device(hidden, encoder_outputs, W, trace=False, **spmd_kwargs):
    nc = _get_nc()
    in_maps = _make_in_maps(hidden, encoder_outputs, W)
    res = run_bass_kernel_spmd(
        nc, in_maps, core_ids=list(range(NCORES)), trace=trace, **spmd_kwargs
    )
    outs = np.concatenate([r["out"] for r in res.results], axis=0)  # [B, S]
    return outs[:, None, :].astype(np.float32), res


def kernel(hidden, encoder_outputs, W, b):
    # `b` (the Linear bias) shifts every energy in a row equally
    # (hidden[b].bias, independent of s), so it cancels in the softmax.
    out, _ = run_device(hidden, encoder_outputs, W)
    return out
